# revision 1
# baseline (speedup 1.0000x reference)
"""Self-contained Trainium2 (Bass/Tile) kernel for nn_CausalSTDiTBlock_80058190397994.

kernel(**inputs) takes the FULL unsharded inputs (x, y, t, tpe, sst, weights)
and returns the full (4, 4096, 1152) float32 output, running SPMD across 8
NeuronCores. Sharding: core = (batch, spatial-half); all AdaLN modulation /
gates / tpe are folded into per-core host-prepped weights and biases.
"""
"""Device kernel builder.

Sharding: core = (b, s-half); per-core tokens tok = t*128 + s_loc (2048 total).
Residual x_res is feature-major (C x 2048) f32, SBUF-resident.
AdaLN modulation, gates, tpe and V-biases are folded into host-prepped
weights/biases (V-bias folds into proj bias since softmax weights sum to 1;
temporal V-bias is per-frame and applied at V eviction via a host tile).

Layouts:
  qT_*/kT_* feature-major (C x ntok) bf16 in DRAM.
  v_* token-major AUGMENTED (ntok x 16*73) bf16: per head h cols [73h, 73h+72] = V,
      col 73h+72 = 1.0 (gives softmax denominators for free in the PV matmul).
  kT_all spatial keys: per frame cols [own 128 | partner 128] (order shared with v_s).
  temporal: virtual order rows/cols v = grp*128 + t*8 + sigma (grp of 8 s_loc).
"""
import sys
sys.path.insert(0, "/opt/trn_rl_repo")
import numpy as np
from contextlib import ExitStack

import concourse.bass as bass
import concourse.mybir as mybir
import concourse.tile as tile
from concourse import bacc
from concourse.masks import make_identity

P = 128
T, C, NH, HD, YL = 16, 1152, 16, 72, 120
S, SH = 256, 128
NTOK = T * SH
GRP, NGRP = 8, 16
NC_C = C // P            # 9
NTT = NTOK // P          # 16
HDA = HD + 1             # 73
CA = NH * HDA            # 1168 augmented V width
bf16 = mybir.dt.bfloat16
f32 = mybir.dt.float32
AF = mybir.ActivationFunctionType
ALU = mybir.AluOpType
SCALE = float(HD) ** -0.5
# V-feature chunks aligned to head blocks: 7 + 7 + 2 heads
VCH = [(0, 7), (7, 14), (14, 16)]


def build(replicate: int = 1):
    nc = bacc.Bacc()
    dp = lambda name, shape, dt: nc.declare_dram_parameter(name, list(shape), dt, isOutput=False)

    xT_own = dp("xT_own", (C, NTOK), f32)
    xT_prt = dp("xT_prt", (C, NTOK), f32)
    yT = dp("yT", (C, YL), bf16)
    wqkv_s = dp("wqkv_s", (C, 3 * C), bf16)
    wproj_s = dp("wproj_s", (C, C), bf16)
    wqkv_t = dp("wqkv_t", (C, 3 * C), bf16)
    wproj_t = dp("wproj_t", (C, C), bf16)
    wq_c = dp("wq_c", (C, C), bf16)
    wk_c = dp("wk_c", (C, C), bf16)
    wv_c = dp("wv_c", (C, C), bf16)
    wproj_c = dp("wproj_c", (C, C), bf16)
    wfc1 = dp("wfc1", (C, 4 * C), bf16)
    wfc2 = dp("wfc2", (4 * C, C), bf16)
    bqkv_s = dp("bqkv_s", (2 * C,), f32)
    bqkv_t = dp("bqkv_t", (P, 2 * C // P * T), f32)  # host-prepped (p, j*T+t)
    bvt = dp("bvt", (P, C), bf16)              # temporal V bias (rows t*8+sig)
    bq_c = dp("bq_c", (C,), f32)
    bk_c = dp("bk_c", (C,), f32)
    bproj_s = dp("bproj_s", (C,), f32)
    bproj_t = dp("bproj_t", (C,), f32)
    bproj_c = dp("bproj_c", (C,), f32)
    bfc1 = dp("bfc1", (4 * C,), f32)
    bfc2 = dp("bfc2", (C,), f32)
    maskneg = dp("maskneg", (P, P), bf16)
    outT = nc.declare_dram_parameter("outT", [C, NTOK], f32, isOutput=True)

    with ExitStack() as ctx:
        tc = ctx.enter_context(tile.TileContext(nc))
        dr = ctx.enter_context(tc.tile_pool(name="dr", bufs=1, space="DRAM"))
        qT_s = dr.tile([C, NTOK], bf16, tag="qT_s")
        kT_all = dr.tile([C, T * S], bf16, tag="kT_all")
        v_s = dr.tile([T * S, C], bf16, tag="v_s")
        qT_t = dr.tile([C, NTOK], bf16, tag="qT_t")
        kT_t = dr.tile([C, NTOK], bf16, tag="kT_t")
        v_t = dr.tile([NTOK, C], bf16, tag="v_t")
        qT_c = dr.tile([C, NTOK], bf16, tag="qT_c")
        kT_y = dr.tile([C, YL], bf16, tag="kT_y")
        v_y = dr.tile([YL, C], bf16, tag="v_y")
        hT = dr.tile([4 * C, NTOK], bf16, tag="hT")
        big = ctx.enter_context(tc.tile_pool(name="big", bufs=1))
        cons = ctx.enter_context(tc.tile_pool(name="cons", bufs=1))
        wp = ctx.enter_context(tc.tile_pool(name="wp", bufs=2))
        rp = ctx.enter_context(tc.tile_pool(name="rp", bufs=4))
        sp = ctx.enter_context(tc.tile_pool(name="sp", bufs=2))
        lnp = ctx.enter_context(tc.tile_pool(name="lnp", bufs=1))
        sqp = ctx.enter_context(tc.tile_pool(name="sqp", bufs=1))
        bcp = ctx.enter_context(tc.tile_pool(name="bcp", bufs=1))
        lrow = ctx.enter_context(tc.tile_pool(name="lrow", bufs=1))
        ap_ = ctx.enter_context(tc.tile_pool(name="ap", bufs=2))
        fr = ctx.enter_context(tc.tile_pool(name="fr", bufs=1))
        pp = ctx.enter_context(tc.tile_pool(name="pp", bufs=2, space="PSUM"))
        pa = ctx.enter_context(tc.tile_pool(name="pa", bufs=1, space="PSUM"))
        pa2 = ctx.enter_context(tc.tile_pool(name="pa2", bufs=2, space="PSUM"))

        # ---------- constants ----------
        ident = cons.tile([P, P], bf16, tag="ident")
        make_identity(nc, ident[:])
        ones_f = cons.tile([P, 1], f32, tag="ones_f")
        nc.vector.memset(ones_f[:], 1.0)
        ones16 = cons.tile([P, 16], bf16, tag="ones16")
        nc.vector.memset(ones16[:], 1.0)
        mask_sb = cons.tile([P, P], bf16, tag="mask")
        nc.sync.dma_start(out=mask_sb[:], in_=maskneg[:, :])
        bvt_sb = cons.tile([P, C], bf16, tag="bvt")
        nc.sync.dma_start(out=bvt_sb[:], in_=bvt[:, :])
        eps_t = cons.tile([1, 1], f32, tag="eps")
        nc.vector.memset(eps_t[:], 1e-6)

        def bias_cols(src, n, tag):
            t_ = cons.tile([P, n // P], f32, tag=tag)
            nc.gpsimd.dma_start(out=t_[:], in_=src.rearrange("(j p) -> p j", p=P))
            return t_
        b_qkv_s = bias_cols(bqkv_s, 2 * C, "b_qkv_s")
        b_q_c = bias_cols(bq_c, C, "b_q_c")
        b_k_c = bias_cols(bk_c, C, "b_k_c")
        b_proj_s = bias_cols(bproj_s, C, "b_proj_s")
        b_proj_t = bias_cols(bproj_t, C, "b_proj_t")
        b_proj_c = bias_cols(bproj_c, C, "b_proj_c")
        b_fc1 = bias_cols(bfc1, 4 * C, "b_fc1")
        b_fc2 = bias_cols(bfc2, C, "b_fc2")
        b_qkv_t = cons.tile([P, 2 * C // P, T], f32, tag="b_qkv_t")
        nc.gpsimd.dma_start(out=b_qkv_t[:], in_=bqkv_t.rearrange("p (j t) -> p j t", t=T))

        x_res = big.tile([P, NC_C, NTOK], f32, tag="xres")

        for rep in range(replicate):
            for i in range(NC_C):
                nc.sync.dma_start(out=x_res[:, i, :], in_=xT_own[i * P:(i + 1) * P, :])

            # =================== LayerNorm (feature-major) ===================
            def layer_norm(src_get, dst):
                """src_get(i, ch) -> f32 AP (128 x 512); dst big [P,NC_C,NTOK] bf16.
                Fully chunk-local: stats, broadcast and apply per 512-token chunk."""
                for ch in range(NTOK // 512):
                    ps1 = pp.tile([1, 512], f32, tag="ps")
                    ps2 = pp.tile([1, 512], f32, tag="ps")
                    for i in range(NC_C):
                        xs = src_get(i, ch)
                        sq = sqp.tile([P, 512], bf16, tag="ln_sq")
                        nc.vector.tensor_mul(sq[:], xs, xs)
                        nc.tensor.matmul(ps1[:], ones_f[:], xs,
                                         start=(i == 0), stop=(i == NC_C - 1))
                        nc.tensor.matmul(ps2[:], ones16[:, 0:1], sq[:],
                                         start=(i == 0), stop=(i == NC_C - 1))
                    ra = lrow.tile([1, 512], f32, tag="ln_a")
                    rb = lrow.tile([1, 512], f32, tag="ln_b")
                    rc = lrow.tile([1, 512], f32, tag="ln_c")
                    rd = lrow.tile([1, 512], bf16, tag="ln_d")
                    nc.vector.tensor_scalar_mul(out=ra[:], in0=ps1[:], scalar1=1.0 / C)  # mu
                    nc.vector.tensor_mul(rc[:], ra[:], ra[:])                            # mu^2
                    nc.vector.scalar_tensor_tensor(out=rb[:], in0=ps2[:], scalar=1.0 / C,
                                                   in1=rc[:], op0=ALU.mult,
                                                   op1=ALU.subtract)                     # var
                    nc.scalar.activation(rb[:], rb[:], AF.Sqrt, bias=eps_t[:])               # sd
                    nc.vector.reciprocal(rc[:], rb[:])                                   # r
                    nc.vector.tensor_mul(rd[:], ra[:], rc[:])                            # mu*r (bf16)
                    rbc = bcp.tile([P, 512], f32, tag="ln_rbc")
                    nc.gpsimd.partition_broadcast(rbc[:], rc[:])
                    mbc = bcp.tile([P, 512], bf16, tag="ln_mbc")
                    nc.gpsimd.partition_broadcast(mbc[:], rd[:])
                    for i in range(NC_C):
                        d = dst[:, i, ch * 512:(ch + 1) * 512]
                        nc.vector.tensor_mul(d, src_get(i, ch), rbc[:])
                        nc.vector.tensor_sub(d, d, mbc[:])

            def src_own(i, ch):
                return x_res[:, i, ch * 512:(ch + 1) * 512]

            def src_prt(i, ch):
                t_ = lnp.tile([P, 512], f32, tag="ln_src")
                nc.sync.dma_start(out=t_[:], in_=xT_prt[i * P:(i + 1) * P,
                                                        ch * 512:(ch + 1) * 512])
                return t_[:]

            x_ln = big.tile([P, NC_C, NTOK], bf16, tag="xact")
            layer_norm(src_own, x_ln)
            x_ln_prt = big.tile([P, NC_C, NTOK], bf16, tag="prt")
            layer_norm(src_prt, x_ln_prt)

            # =================== feature-major projection ===================
            def proj_fm(w_dram, rhs_get, m_tiles, evict_fn, wcol0=0, n_tok=NTOK):
                nch = (n_tok + 511) // 512
                for m in range(m_tiles):
                    wt = wp.tile([P, NC_C, P], bf16, tag="w")
                    nc.sync.dma_start(
                        out=wt[:],
                        in_=w_dram[:, wcol0 + m * P: wcol0 + (m + 1) * P]
                            .rearrange("(k p) m -> p k m", p=P))
                    for ch in range(nch):
                        cw = min(512, n_tok - ch * 512)
                        ps = pp.tile([P, 512], f32, tag="ps")
                        for k in range(NC_C):
                            nc.tensor.matmul(ps[:, :cw], wt[:, k, :],
                                             rhs_get(k, ch, cw),
                                             start=(k == 0), stop=(k == NC_C - 1))
                        evict_fn(m, ch, ps, cw)

            def ev_plain(dram, bias_t, m, ch, ps, cw, frame_bias=None, fb_off=0):
                st = sp.tile([P, 512], bf16, tag="st")
                if frame_bias is not None:
                    for f4 in range(4):
                        t_ = ch * 4 + f4
                        nc.scalar.activation(st[:, f4 * P:(f4 + 1) * P],
                                             ps[:, f4 * P:(f4 + 1) * P], AF.Identity,
                                             bias=frame_bias[:, fb_off + m, t_:t_ + 1])
                else:
                    nc.scalar.activation(st[:, :cw], ps[:, :cw], AF.Identity,
                                         bias=bias_t[:, m:m + 1])
                nc.gpsimd.dma_start(out=dram[m * P:(m + 1) * P, ch * 512:ch * 512 + cw],
                                  in_=st[:, :cw])

            # ---- spatial Q (own) ----
            rhs_xln = lambda k, ch, cw: x_ln[:, k, ch * 512:ch * 512 + cw]
            rhs_prt = lambda k, ch, cw: x_ln_prt[:, k, ch * 512:ch * 512 + cw]
            proj_fm(wqkv_s, rhs_xln, NC_C,
                    lambda m, ch, ps, cw: ev_plain(qT_s, b_qkv_s, m, ch, ps, cw))

            # ---- spatial K own/prt -> kT_all cols [own|prt] per frame ----
            k_all_r = kT_all.rearrange("c (t s) -> c t s", t=T)

            def ev_k(side):
                def ev(m, ch, ps, cw):
                    st = sp.tile([P, 512], bf16, tag="st")
                    nc.scalar.activation(st[:, :cw], ps[:, :cw], AF.Identity,
                                         bias=b_qkv_s[:, NC_C + m:NC_C + m + 1])
                    nc.gpsimd.dma_start(
                        out=k_all_r[m * P:(m + 1) * P, ch * 4:(ch + 1) * 4,
                                    side * SH:(side + 1) * SH],
                        in_=st[:].rearrange("p (t s) -> p t s", s=SH))
                return ev
            proj_fm(wqkv_s, rhs_xln, NC_C, ev_k(0), wcol0=C)
            proj_fm(wqkv_s, rhs_prt, NC_C, ev_k(1), wcol0=C)

            # ---- token-major V projection into augmented layout ----
            def proj_v(w_dram, wcol0, lhs_of, m_tiles, dst_row_of, vbias=None,
                       mrows=P):
                """lhs_of(m) -> list over k of (128 x mrows) lhsT APs;
                dst_row_of(m) -> (dram_tensor, row0)."""
                for nch in range(3):
                    c0, cw = nch * 512, min(512, C - nch * 512)
                    wt = wp.tile([P, NC_C, 512], bf16, tag="w")
                    nc.sync.dma_start(
                        out=wt[:, :, :cw],
                        in_=w_dram[:, wcol0 + c0: wcol0 + c0 + cw]
                            .rearrange("(k p) m -> p k m", p=P))
                    for m in range(m_tiles):
                        ps = pp.tile([P, 512], f32, tag="ps")
                        lhs = lhs_of(m)
                        for k in range(NC_C):
                            nc.tensor.matmul(ps[:mrows, :cw], lhs[k], wt[:, k, :cw],
                                             start=(k == 0), stop=(k == NC_C - 1))
                        st = sp.tile([P, 512], bf16, tag="st")
                        if vbias is None:
                            nc.scalar.activation(st[:mrows, :cw], ps[:mrows, :cw], AF.Copy)
                        else:
                            nc.vector.tensor_add(st[:mrows, :cw], ps[:mrows, :cw],
                                                 vbias[:mrows, c0:c0 + cw])
                        dram, row0 = dst_row_of(m)
                        nc.gpsimd.dma_start(
                            out=dram[row0:row0 + mrows, c0:c0 + cw],
                            in_=st[:mrows, :cw])

            def lhs_xln(m):
                return [x_ln[:, k, m * P:(m + 1) * P] for k in range(NC_C)]

            def lhs_prt(m):
                return [x_ln_prt[:, k, m * P:(m + 1) * P] for k in range(NC_C)]

            proj_v(wqkv_s, 2 * C, lhs_xln, NTT, lambda m: (v_s, m * S))
            proj_v(wqkv_s, 2 * C, lhs_prt, NTT, lambda m: (v_s, m * S + SH))

            # =================== attention ===================
            def attention(oT_dst, nk, q_get, k_get, v_get, masked, n_qb=NTT):
                nkt = (nk + P - 1) // P
                for qb in range(n_qb):
                    vt_f = fr.tile([P, nkt, NH, HD], bf16, tag="vt_f")
                    nc.sync.dma_start(out=vt_f[:nk if nkt == 1 else P], in_=v_get(qb))
                    qt_f = fr.tile([HD, NH, P], bf16, tag="qt_f")
                    qsrc = q_get(qb)
                    if isinstance(qsrc, list):
                        for h in range(NH):
                            (nc.gpsimd if h % 2 else nc.sync).dma_start(
                                out=qt_f[:, h, :], in_=qsrc[h])
                    else:
                        nc.sync.dma_start(out=qt_f[:], in_=qsrc)
                    o_acc = ap_.tile([P, C], bf16, tag="o_acc")
                    for h in range(NH):
                        kt_t_ = ap_.tile([HD, nkt * P], bf16, tag="a_k")
                        (nc.gpsimd if h % 2 else nc.sync).dma_start(
                            out=kt_t_[:, :nk], in_=k_get(qb, h))
                        e_tiles = []
                        for kt in range(nkt):
                            kp = min(P, nk - kt * P)
                            sps = (pa2 if kt == 0 else pa).tile([P, P], f32, tag="sc%d" % kt)
                            nc.tensor.matmul(sps[:kp, :], kt_t_[:, kt * P:kt * P + kp],
                                             qt_f[:, h, :], start=True, stop=True)
                            e = ap_.tile([P, P], bf16, tag="a_e%d" % kt)
                            if masked:
                                scm = ap_.tile([P, P], f32, tag="a_scm")
                                nc.vector.scalar_tensor_tensor(
                                    out=scm[:kp, :], in0=sps[:kp, :], scalar=SCALE,
                                    in1=mask_sb[:kp, :], op0=ALU.mult, op1=ALU.add)
                                nc.scalar.activation(e[:kp, :], scm[:kp, :], AF.Exp)
                            else:
                                nc.scalar.activation(e[:kp, :], sps[:kp, :], AF.Exp,
                                                     scale=SCALE)
                            e_tiles.append((e, kp))
                        o_ps = pa.tile([P, HD], f32, tag="o")
                        o2_ps = pa.tile([P, 1], f32, tag="o2")
                        for kt, (e, kp) in enumerate(e_tiles):
                            nc.tensor.matmul(o_ps[:], e[:kp, :],
                                             vt_f[:kp, kt, h, :],
                                             start=(kt == 0), stop=(kt == nkt - 1))
                            nc.tensor.matmul(o2_ps[:], e[:kp, :],
                                             ones16[:kp, 0:1],
                                             start=(kt == 0), stop=(kt == nkt - 1))
                        rec = ap_.tile([P, 1], f32, tag="a_rec")
                        nc.vector.reciprocal(rec[:], o2_ps[:])
                        nc.vector.tensor_scalar_mul(out=o_acc[:, h * HD:(h + 1) * HD],
                                                    in0=o_ps[:], scalar1=rec[:])
                    for cb in range(NC_C):
                        tp = pa.tile([P, P], bf16, tag="tp")
                        nc.tensor.transpose(tp[:], o_acc[:, cb * P:(cb + 1) * P], ident[:])
                        nc.scalar.copy(oT_dst[:, cb, qb * P:(qb + 1) * P], tp[:])

            # ---- spatial attention ----
            q_s_r = qT_s.rearrange("(h j) (t s) -> j h t s", j=HD, t=T)
            k_sr = kT_all.rearrange("(h j) (t s) -> j h t s", j=HD, t=T)
            v_sr = v_s.rearrange("(t k p) (h d) -> t p k h d", t=T, p=P, d=HD)
            oT_sp = big.tile([P, NC_C, NTOK], bf16, tag="prt")
            attention(
                oT_sp, S,
                q_get=lambda qb: q_s_r[:, :, qb, :],
                k_get=lambda qb, h: k_sr[:, h, qb, :],
                v_get=lambda qb: v_sr[qb],
                masked=False)

            # ---- residual projection (feature-major into x_res) ----
            def proj_residual(w_dram, rhs_big, bias_t, scatter=False):
                for m in range(NC_C):
                    wt = wp.tile([P, NC_C, P], bf16, tag="w")
                    nc.sync.dma_start(out=wt[:],
                                      in_=w_dram[:, m * P:(m + 1) * P]
                                          .rearrange("(k p) m -> p k m", p=P))
                    for ch in range(NTOK // 512):
                        ps = pp.tile([P, 512], f32, tag="ps")
                        for k in range(NC_C):
                            nc.tensor.matmul(ps[:], wt[:, k, :],
                                             rhs_big[:, k, ch * 512:(ch + 1) * 512],
                                             start=(k == 0), stop=(k == NC_C - 1))
                        if not scatter:
                            nc.vector.scalar_tensor_tensor(
                                out=x_res[:, m, ch * 512:(ch + 1) * 512],
                                in0=ps[:], scalar=bias_t[:, m:m + 1],
                                in1=x_res[:, m, ch * 512:(ch + 1) * 512],
                                op0=ALU.add, op1=ALU.add)
                        else:
                            xr = x_res[:, m, :].rearrange("p (t s) -> p t s", t=T)
                            for g4 in range(4):
                                g = ch * 4 + g4
                                nc.vector.scalar_tensor_tensor(
                                    out=xr[:, :, g * GRP:(g + 1) * GRP],
                                    in0=ps[:, g4 * P:(g4 + 1) * P]
                                        .rearrange("p (t s) -> p t s", s=GRP),
                                    scalar=bias_t[:, m:m + 1],
                                    in1=xr[:, :, g * GRP:(g + 1) * GRP],
                                    op0=ALU.add, op1=ALU.add)

            proj_residual(wproj_s, oT_sp, b_proj_s)

            # =================== temporal ===================
            x_bf = big.tile([P, NC_C, NTOK], bf16, tag="xact")
            for i in range(NC_C):
                nc.vector.tensor_copy(out=x_bf[:, i, :], in_=x_res[:, i, :])
            rhs_xbf = lambda k, ch, cw: x_bf[:, k, ch * 512:ch * 512 + cw]
            proj_fm(wqkv_t, rhs_xbf, NC_C,
                    lambda m, ch, ps, cw: ev_plain(qT_t, None, m, ch, ps, cw,
                                                   frame_bias=b_qkv_t, fb_off=0))
            proj_fm(wqkv_t, rhs_xbf, NC_C,
                    lambda m, ch, ps, cw: ev_plain(kT_t, None, m, ch, ps, cw,
                                                   frame_bias=b_qkv_t, fb_off=NC_C),
                    wcol0=C)

            # stage x_bf into temporal-virtual token order (contiguous lhsT)
            x_virt = big.tile([P, NC_C, NTOK], bf16, tag="prt")
            for k in range(NC_C):
                xrk = x_bf[:, k, :].rearrange("p (t s) -> p t s", t=T)
                xvk = x_virt[:, k, :].rearrange("p (g v) -> p g v", g=NGRP)
                for g in range(NGRP):
                    nc.vector.tensor_copy(
                        out=xvk[:, g, :],
                        in_=xrk[:, :, g * GRP:(g + 1) * GRP])

            def lhs_virt(g):
                return [x_virt[:, k, g * P:(g + 1) * P] for k in range(NC_C)]

            proj_v(wqkv_t, 2 * C, lhs_virt, NGRP, lambda g: (v_t, g * P),
                   vbias=bvt_sb)

            q_t_r = qT_t.rearrange("(h j) (t s) -> j h t s", j=HD, t=T)
            k_t_r = kT_t.rearrange("(h j) (t s) -> j h t s", j=HD, t=T)
            v_t_r = v_t.rearrange("(g p) (h d) -> g p h d", p=P, d=HD)
            oT_t = big.tile([P, NC_C, NTOK], bf16, tag="prt")
            attention(
                oT_t, P,
                q_get=lambda qb: [q_t_r[:, h, :, qb * GRP:(qb + 1) * GRP] for h in range(NH)],
                k_get=lambda qb, h: k_t_r[:, h, :, qb * GRP:(qb + 1) * GRP],
                v_get=lambda qb: v_t_r[qb].rearrange("p (o h) d -> p o h d", o=1),
                masked=True, n_qb=NGRP)
            proj_residual(wproj_t, oT_t, b_proj_t, scatter=True)

            # =================== cross ===================
            x_bf2 = big.tile([P, NC_C, NTOK], bf16, tag="xact")
            for i in range(NC_C):
                nc.vector.tensor_copy(out=x_bf2[:, i, :], in_=x_res[:, i, :])
            rhs_xbf2 = lambda k, ch, cw: x_bf2[:, k, ch * 512:ch * 512 + cw]
            proj_fm(wq_c, rhs_xbf2, NC_C,
                    lambda m, ch, ps, cw: ev_plain(qT_c, b_q_c, m, ch, ps, cw))

            y_sb = fr.tile([P, NC_C, YL], bf16, tag="y_sb")
            for k in range(NC_C):
                nc.sync.dma_start(out=y_sb[:, k, :], in_=yT[k * P:(k + 1) * P, :])
            rhs_y = lambda k, ch, cw: y_sb[:, k, :]
            proj_fm(wk_c, rhs_y, NC_C,
                    lambda m, ch, ps, cw: ev_plain(kT_y, b_k_c, m, ch, ps, cw),
                    n_tok=YL)

            def lhs_y(m):
                return [y_sb[:, k, :] for k in range(NC_C)]
            proj_v(wv_c, 0, lhs_y, 1, lambda m: (v_y, 0), mrows=YL)

            q_c_r = qT_c.rearrange("(h j) (t s) -> j h t s", j=HD, t=T)
            k_y_r = kT_y.rearrange("(h j) n -> j h n", j=HD)
            oT_c = big.tile([P, NC_C, NTOK], bf16, tag="prt")
            attention(
                oT_c, YL,
                q_get=lambda qb: q_c_r[:, :, qb, :],
                k_get=lambda qb, h: k_y_r[:, h, :],
                v_get=lambda qb: v_y.rearrange("(o n) (h d) -> n o h d", o=1, d=HD),
                masked=False)
            proj_residual(wproj_c, oT_c, b_proj_c)

            # =================== MLP ===================
            x_ln2 = big.tile([P, NC_C, NTOK], bf16, tag="xact")
            layer_norm(src_own, x_ln2)
            rhs_xln2 = lambda k, ch, cw: x_ln2[:, k, ch * 512:ch * 512 + cw]

            def ev_gelu(m, ch, ps, cw):
                st = sp.tile([P, 512], bf16, tag="st")
                nc.scalar.activation(st[:, :cw], ps[:, :cw], AF.Gelu_apprx_tanh,
                                     bias=b_fc1[:, m:m + 1])
                nc.gpsimd.dma_start(out=hT[m * P:(m + 1) * P, ch * 512:ch * 512 + cw],
                                  in_=st[:, :cw])
            proj_fm(wfc1, rhs_xln2, 4 * C // P, ev_gelu)

            for m in range(NC_C):
                wt = wp.tile([P, 4 * C // P, P], bf16, tag="w")
                nc.sync.dma_start(out=wt[:], in_=wfc2[:, m * P:(m + 1) * P]
                                  .rearrange("(k p) m -> p k m", p=P))
                for ch in range(NTOK // 512):
                    ps = pp.tile([P, 512], f32, tag="ps")
                    for k2 in range(4 * C // P // 2):
                        rh = rp.tile([P, 2, 512], bf16, tag="rh")
                        eng = (nc.sync, nc.gpsimd, nc.scalar)[k2 % 3]
                        eng.dma_start(out=rh[:],
                                      in_=hT[k2 * 2 * P:(k2 + 1) * 2 * P,
                                             ch * 512:(ch + 1) * 512]
                                          .rearrange("(o p) n -> p o n", p=P))
                        for kk in range(2):
                            k = k2 * 2 + kk
                            nc.tensor.matmul(ps[:], wt[:, k, :], rh[:, kk, :],
                                             start=(k == 0), stop=(k == 4 * C // P - 1))
                    nc.vector.scalar_tensor_tensor(
                        out=x_res[:, m, ch * 512:(ch + 1) * 512],
                        in0=ps[:], scalar=b_fc2[:, m:m + 1],
                        in1=x_res[:, m, ch * 512:(ch + 1) * 512],
                        op0=ALU.add, op1=ALU.add)

            for i in range(NC_C):
                nc.sync.dma_start(out=outT[i * P:(i + 1) * P, :], in_=x_res[:, i, :])

    nc.finalize()
    return nc


# ======================= SPMD runner =======================
import time
import jax
from jax.sharding import Mesh, PartitionSpec
from jax.experimental.shard_map import shard_map
from concourse.bass2jax import _bass_exec_p, install_neuronx_cc_hook, partition_id_tensor

def make_runner(nc: bass.Bass, n_cores: int = 8):
    install_neuronx_cc_hook()
    assert nc.dbg_addr is None or not nc.dbg_callbacks

    partition_name = nc.partition_id_tensor.name if nc.partition_id_tensor else None
    in_names, out_names, out_avals, zero_outs = [], [], [], []
    for alloc in nc.m.functions[0].allocations:
        if not isinstance(alloc, mybir.MemoryLocationSet):
            continue
        name = alloc.memorylocations[0].name
        if alloc.kind == "ExternalInput":
            if name != partition_name:
                in_names.append(name)
        elif alloc.kind == "ExternalOutput":
            out_names.append(name)
            shape = tuple(alloc.tensor_shape)
            dtype = mybir.dt.np(alloc.dtype)
            out_avals.append(jax.core.ShapedArray(shape, dtype))
            zero_outs.append(np.zeros(shape, dtype))
    n_params = len(in_names)
    n_outs = len(out_avals)
    all_in_names = list(in_names) + list(out_names)
    if partition_name is not None:
        all_in_names.append(partition_name)

    def _body(*args):
        operands = list(args)
        if partition_name is not None:
            operands.append(partition_id_tensor())
        outs = _bass_exec_p.bind(
            *operands,
            out_avals=tuple(out_avals),
            in_names=tuple(all_in_names),
            out_names=tuple(out_names),
            lowering_input_output_aliases=(),
            sim_require_finite=True,
            sim_require_nnan=True,
            nc=nc,
        )
        return tuple(outs)

    devices = jax.devices()[:n_cores]
    mesh = Mesh(np.asarray(devices), ("core",))
    in_specs = (PartitionSpec("core"),) * (n_params + n_outs)
    out_specs = (PartitionSpec("core"),) * n_outs
    donate = tuple(range(n_params, n_params + n_outs))
    sharded = jax.jit(
        shard_map(_body, mesh=mesh, in_specs=in_specs, out_specs=out_specs,
                  check_rep=False),
        donate_argnums=donate, keep_unused=True,
    )

    sharding = jax.sharding.NamedSharding(mesh, PartitionSpec("core"))

    def run(in_maps, n_iters=3):
        per_core = [[np.asarray(m[name]) for name in in_names] for m in in_maps]
        concat_in = [
            np.concatenate([per_core[c][i] for c in range(n_cores)], axis=0)
            for i in range(n_params)
        ]
        dev_in = [jax.device_put(a, sharding) for a in concat_in]
        times = []
        out_arrs = None
        for it in range(n_iters):
            dev_zeros = [
                jax.device_put(np.zeros((n_cores * z.shape[0], *z.shape[1:]), z.dtype),
                               sharding)
                for z in zero_outs
            ]
            for z in dev_zeros:
                z.block_until_ready()
            t0 = time.perf_counter()
            out = sharded(*dev_in, *dev_zeros)
            for o in out:
                o.block_until_ready()
            t1 = time.perf_counter()
            times.append(t1 - t0)
            out_arrs = out
        results = [
            {
                name: np.asarray(out_arrs[i]).reshape(n_cores, *out_avals[i].shape)[c]
                for i, name in enumerate(out_names)
            }
            for c in range(n_cores)
        ]
        return results, times

    return run


# ======================= host prep + entry point =======================
import ml_dtypes

B = 4
bfloat16 = ml_dtypes.bfloat16


def _bf(x):
    return np.ascontiguousarray(x, dtype=np.float32).astype(bfloat16)


def build_in_maps(inputs):
    x = np.asarray(inputs['x'], np.float32)
    y = np.asarray(inputs['y'], np.float32)
    t = np.asarray(inputs['t'], np.float32)
    tpe = np.asarray(inputs['tpe'], np.float32)
    sst = np.asarray(inputs['sst'], np.float32)
    W = {k: np.asarray(inputs[k], np.float32) for k in inputs
         if k not in ('x', 'y', 't', 'tpe', 'sst')}

    t6 = sst[None] + t.reshape(B, 6, C)
    sh_msa, sc_msa, g_msa, sh_mlp, sc_mlp, g_mlp = [t6[:, i] for i in range(6)]

    mask = np.zeros((P, P), np.float32)
    t2 = np.arange(P)[:, None] // GRP
    s2 = np.arange(P)[:, None] % GRP
    t1 = np.arange(P)[None, :] // GRP
    s1 = np.arange(P)[None, :] % GRP
    mask[~((s2 == s1) & (t2 <= t1))] = -30000.0

    in_maps = []
    for b in range(B):
        wqkv_s = W['qkv_s_w'] * (1.0 + sc_msa[b])[None, :]
        bqkv_s = W['qkv_s_w'] @ sh_msa[b] + W['qkv_s_b']
        wproj_s = W['proj_s_w'] * g_msa[b][:, None]
        bproj_s = g_msa[b] * W['proj_s_b'] + wproj_s @ bqkv_s[2 * C:]
        bqkv_t_frames = tpe[0] @ W['qkv_t_w'].T + W['qkv_t_b'][None]    # (T, 3C)
        wproj_t = W['proj_t_w'] * g_msa[b][:, None]
        bproj_t = g_msa[b] * W['proj_t_b']
        bk_c = W['kv_c_b'][:C]
        bv_c = W['kv_c_b'][C:]
        bproj_c = W['proj_c_b'] + W['proj_c_w'] @ bv_c
        wfc1 = W['fc1_w'] * (1.0 + sc_mlp[b])[None, :]
        bfc1 = W['fc1_w'] @ sh_mlp[b] + W['fc1_b']
        wfc2 = W['fc2_w'] * g_mlp[b][:, None]
        bfc2 = g_mlp[b] * W['fc2_b']

        # per-frame qkv_t Q,K bias in device layout (p, j*T + t)
        bqt = bqkv_t_frames[:, :2 * C]                       # (T, 2C)
        bqt_dev = bqt.T.reshape(2 * C // P, P, T)            # (j, p, t)
        bqt_dev = np.ascontiguousarray(bqt_dev.transpose(1, 0, 2)).reshape(P, -1)
        # temporal V bias tile: partition p = t*8+sig -> bias row t, (128, C)
        bvt_tile = np.repeat(bqkv_t_frames[:, 2 * C:], GRP, axis=0)    # (128, C)

        common = dict(
            yT=_bf(y[b].T),
            wqkv_s=_bf(wqkv_s.T), wproj_s=_bf(wproj_s.T),
            wqkv_t=_bf(W['qkv_t_w'].T), wproj_t=_bf(wproj_t.T),
            wq_c=_bf(W['q_c_w'].T), wk_c=_bf(W['kv_c_w'][:C].T),
            wv_c=_bf(W['kv_c_w'][C:].T), wproj_c=_bf(W['proj_c_w'].T),
            wfc1=_bf(wfc1.T), wfc2=_bf(wfc2.T),
            bqkv_s=np.ascontiguousarray(bqkv_s[:2 * C], np.float32),
            bqkv_t=np.ascontiguousarray(bqt_dev, np.float32),
            bvt=_bf(bvt_tile),
            bq_c=np.ascontiguousarray(W['q_c_b'], np.float32),
            bk_c=np.ascontiguousarray(bk_c, np.float32),
            bproj_s=np.ascontiguousarray(bproj_s, np.float32),
            bproj_t=np.ascontiguousarray(bproj_t, np.float32),
            bproj_c=np.ascontiguousarray(bproj_c, np.float32),
            bfc1=np.ascontiguousarray(bfc1, np.float32),
            bfc2=np.ascontiguousarray(bfc2, np.float32),
            maskneg=_bf(mask),
        )
        xb = x[b].reshape(T, S, C)
        for sh in range(2):
            own = xb[:, sh * SH:(sh + 1) * SH, :].reshape(NTOK, C)
            prt = xb[:, (1 - sh) * SH:(2 - sh) * SH, :].reshape(NTOK, C)
            m = dict(common)
            m['xT_own'] = np.ascontiguousarray(own.T, np.float32)
            m['xT_prt'] = np.ascontiguousarray(prt.T, np.float32)
            in_maps.append(m)
    return in_maps


def assemble(outs):
    xout = np.zeros((B, T * S, C), np.float32)
    ci = 0
    for b in range(B):
        for sh in range(2):
            o = outs[ci]['outT']            # (C, NTOK)
            tok = o.T.reshape(T, SH, C)
            xout[b].reshape(T, S, C)[:, sh * SH:(sh + 1) * SH, :] = tok
            ci += 1
    return xout


_CACHE = {}


def run_kernel(inputs, replicate=1, n_iters=2):
    key = replicate
    if key not in _CACHE:
        nc = build(replicate)
        _CACHE[key] = make_runner(nc, 8)
    run = _CACHE[key]
    in_maps = build_in_maps(inputs)
    results, times = run(in_maps, n_iters=n_iters)
    return assemble(results), times


def kernel(**inputs):
    out, _ = run_kernel(inputs, replicate=1, n_iters=1)
    return out

